# revision 50
# baseline (speedup 1.0000x reference)
"""Trainium2 Bass kernel for nn_Baseline_635655160228 (retrieval_knn).

Reference computation (B=64, WAYS=10, SHOTS=5, C=128, H=W=32):
    cov_j = centered-Gram(support_j) / (N-1)          # [ways, C, C], N = shots*hw
    qn    = q / ||q||_2(per channel row)              # [B, C, hw]
    sim[b,j,p] = qn_p^T cov_j qn_p                    # diag quadratic form
    out[b,j]   = sum_p leaky_relu(sim) * conv_w[p]

Algebraic restructuring:
  cov_j is PSD, so LeakyReLU is the identity, and
      out[b,j] = <cov_j, W_b>_F,   W_b = qn diag(w') qn^T  (w' = conv_w/(N-1))
  Since the channel-row normalization is a per-channel scale (qn = diag(rin) q),
  W_b = diag(rin_b) W~_b diag(rin_b) with W~_b built from RAW q.  W~ is
  symmetric, so the two-sided scale is: ACT-scale rows, PE-transpose,
  ACT-scale rows again.
  Mean correction applied at the end:
      out[b,j] = <R_j, W_b> - (1/N) m_j^T W_b m_j   (R raw Gram, m row sums).

Distribution over 8 NeuronCores:
  - data-parallel over the query batch (8 queries per core)
  - Grams sharded over the pixel axis (128-pixel slice per core), combined
    with one bf16 AllReduce of raw Gram + row sums, overlapped with all the
    query-side work.  A tiny warm-up AllReduce at t~0 absorbs comm-init.

Layout is precomputed on host (pure shard/cast/transpose, no math):
  - support arrives pre-transposed as xts [p=128, ways*shots, C+1] bf16 with a
    ones column (row sums fall out of the Gram matmul)
  - q arrives both as qT [p, ci, b, c] bf16 (for the W~ build) and natural
    [c, b, hw] bf16 (for the row norms)
  Host-side prep kills all 114 PE transposes and the per-way rearranging DMAs.

Frobenius stage is k-packed: 12 c-columns per matmul (lhsT [128, 120],
rhs [128, 96]), PSUM-accumulated over 11 chunks; the 12 diagonal [10,8]
blocks are summed on DVE/ACT.  11 matmuls instead of 128.
"""

import os

import numpy as np
import ml_dtypes

os.environ.setdefault("NEURON_RT_DBG_RDH_CC", "0")

B, WAYS, SHOTS, C, H, W = 64, 10, 5, 128, 32, 32
HW = H * W                       # 1024
NCORES = 8
BLOC = B // NCORES               # 8 queries per core
PIX = HW // NCORES               # 128-pixel support slice per core
NTOT = SHOTS * HW                # 5120 samples per way
DENOM = float(NTOT - 1)          # 5119
CHUNKS = WAYS * SHOTS            # 50 local [128, C] sample chunks
QCH = HW // 128                  # 8 pixel chunks per query
KP = 12                          # Frobenius column-pack factor
NFB = (C + KP - 1) // KP         # 11 packed Frobenius matmuls

_CACHE = {}


def _build_program():
    import concourse.bass as bass
    import concourse.tile as tile
    from concourse import bacc, mybir

    f32 = mybir.dt.float32
    bf16 = mybir.dt.bfloat16
    AF = mybir.ActivationFunctionType
    ALU = mybir.AluOpType

    nc = bacc.Bacc("TRN2", target_bir_lowering=False, debug=False,
                   num_devices=NCORES)

    # host-prepped inputs (bf16, pre-transposed layouts)
    xts_d = nc.dram_tensor("support", [128, CHUNKS, C + 1], bf16,
                           kind="ExternalInput")
    qT_d = nc.dram_tensor("q", [128, QCH, BLOC, C], bf16,
                          kind="ExternalInput")
    qnat_d = nc.dram_tensor("qnat", [C, BLOC, HW], bf16,
                            kind="ExternalInput")
    w_d = nc.dram_tensor("conv_w", [HW], f32, kind="ExternalInput")
    out_d = nc.dram_tensor("out", [WAYS, BLOC], f32, kind="ExternalOutput")

    # collective bounce buffers (transposed layout: [c, c'-col, way])
    cc_in = nc.dram_tensor("cc_in", [C, C + 1, WAYS], bf16)
    cc_out = nc.dram_tensor("cc_out", [C, C + 1, WAYS], bf16,
                            addr_space="Shared")
    # warm-up collective scratch: never initialized nor read — its only
    # purpose is to ring the CC doorbell as early as possible so the
    # CC-stream init / rendezvous runs under the local compute.  AllGather:
    # lowest-floor collective (~4.6us vs ~9.7us for AllReduce).
    ccw_in = nc.dram_tensor("ccw_in", [32, 8], bf16)
    ccw_out = nc.dram_tensor("ccw_out", [128, 8], bf16)
    groups = [list(range(NCORES))]

    with tile.TileContext(nc) as tc:
        # warm-up collective first — no data deps, doorbell rings at t~0.
        # Pairwise groups: shortest possible flight for the stream warm-up.
        from concourse.mybir import AluOpType as _ALU
        nc.gpsimd.collective_compute(
            "AllGather", _ALU.bypass,
            replica_groups=[[0, 1, 2, 3], [4, 5, 6, 7]],
            ins=[ccw_in[:]], outs=[ccw_out[:]],
        )
        with (
            tc.tile_pool(name="const", bufs=1) as constp,
            tc.tile_pool(name="big", bufs=1) as big,
            tc.tile_pool(name="vt", bufs=2) as vtp,
            tc.tile_pool(name="tp_ps", bufs=2, space="PSUM") as tp_ps,
            tc.tile_pool(name="gram_ps", bufs=2, space="PSUM") as gram_ps,
            tc.tile_pool(name="w_ps", bufs=2, space="PSUM") as w_ps,
            tc.tile_pool(name="fr_ps", bufs=1, space="PSUM") as fr_ps,
        ):
            ident_d = nc.inline_tensor(
                np.eye(128, dtype=ml_dtypes.bfloat16), name="ident_const")
            ident = constp.tile([128, 128], bf16, tag="ident")

            # Frobenius diag-block mask M[(i,j),(i',b)] = (i == i') and the
            # partition-block fold selector SEL[(i,j), j] = 1
            msk_np = np.zeros((KP * WAYS, KP * BLOC), np.float32)
            sel_np = np.zeros((KP * WAYS, WAYS), np.float32)
            for i in range(KP):
                msk_np[i * WAYS:(i + 1) * WAYS,
                       i * BLOC:(i + 1) * BLOC] = 1.0
                for j in range(WAYS):
                    sel_np[i * WAYS + j, j] = 1.0
            msk_d = nc.inline_tensor(msk_np, name="fmask_const")
            msk = constp.tile([KP * WAYS, KP, BLOC], f32, tag="msk")
            sel_d = nc.inline_tensor(sel_np, name="sel_const")
            sel = constp.tile([KP * WAYS, WAYS], f32, tag="sel")

            wp = constp.tile([128, QCH], f32, tag="wp")        # conv_w p-major
            wps = constp.tile([128, QCH], f32, tag="wps")      # conv_w/(N-1)

            # ---------------- persistent tensors ----------------
            # R and W live in transposed layouts [c, c'-col, way/query] so the
            # k-packed Frobenius sees its (ci, j)/(ci, b) column groups as one
            # CONTIGUOUS free dim (matmul APs allow only one free dim).
            # Both are padded to KP*NFB=132 c'-cols (zeroed); rall's col C is
            # the mean column and pairs with wsb's zero pad -> contributes 0.
            CP = KP * NFB
            xts = big.tile([128, CHUNKS, C + 1], bf16, tag="xts")
            rpart = big.tile([C, C + 1, WAYS], bf16, tag="rpart")
            rall = big.tile([C, CP, WAYS], bf16, tag="rall")
            qT = big.tile([128, QCH, BLOC, C], bf16, tag="qT")
            wqT = big.tile([128, QCH, BLOC, C], bf16, tag="wqT")
            qnat = big.tile([C, BLOC, HW], bf16, tag="qnat")
            wsb = big.tile([C, CP, BLOC], bf16, tag="wsb")

            nsq = constp.tile([128, BLOC], f32, tag="nsq")
            rin = constp.tile([128, BLOC], f32, tag="rin")
            tnw = constp.tile([128, BLOC], f32, tag="tnw")
            mallN = constp.tile([C, WAYS], bf16, tag="mallN")
            msT = constp.tile([WAYS, C], f32, tag="msT")
            ytmp = constp.tile([WAYS, BLOC, C], f32, tag="ytmp")
            ysb = constp.tile([WAYS, BLOC], f32, tag="ysb")
            masked = constp.tile([KP * WAYS, KP, BLOC], f32, tag="masked")
            red = constp.tile([KP * WAYS, BLOC], f32, tag="red")
            fin = constp.tile([WAYS, BLOC], f32, tag="fin")

            nc.vector.memset(rall[:, C + 1:CP, :], 0.0)
            nc.vector.memset(wsb[:, C:CP, :], 0.0)

            # ---------------- input DMAs ----------------
            # sync: ident + first half of xts; scalar: rest of xts + qT;
            # gpsimd (SWDGE): qnat.  Support path has priority end-to-end.
            nc.sync.dma_start(ident[:], ident_d[:])
            for i in range(3):
                j0, j1 = [(0, 10), (10, 20), (20, 25)][i]
                nc.sync.dma_start(xts[:, j0:j1, :], xts_d[:, j0:j1, :])
            for i in range(3):
                j0, j1 = [(25, 35), (35, 45), (45, 50)][i]
                nc.scalar.dma_start(xts[:, j0:j1, :], xts_d[:, j0:j1, :])
            for h in range(2):
                nc.gpsimd.dma_start(qnat[:, 4 * h:4 * h + 4, :],
                                    qnat_d[:, 4 * h:4 * h + 4, :])
            for h in range(2):
                nc.scalar.dma_start(qT[:, 4 * h:4 * h + 4, :, :],
                                    qT_d[:, 4 * h:4 * h + 4, :, :])
            nc.sync.dma_start(wp[:], w_d.rearrange("(ci p) -> p ci", p=128))
            nc.vector.tensor_scalar_mul(wps[:], wp[:], 1.0 / DENOM)
            nc.gpsimd.dma_start(msk[:], msk_d.rearrange("p (ci b) -> p ci b",
                                                        b=BLOC))
            nc.gpsimd.dma_start(sel[:], sel_d[:])

            # ---------------- PE warm-up (p-state ramp) ----------------
            warm = fr_ps.tile([128, 128], f32, tag="fr")
            last_warm = None
            for wi in range(16):
                last_warm = nc.tensor.matmul(
                    warm[:], lhsT=ident[:], rhs=ident[:],
                    start=(wi == 0), stop=(wi == 15))

            # ---------------- stage S: local support Grams ----------------
            for j in range(WAYS):
                base = SHOTS * j
                gp = gram_ps.tile([C, C + 1], f32, tag="gram")
                for t in range(SHOTS):
                    g_ = nc.tensor.matmul(
                        gp[:], lhsT=xts[:, base + t, 0:C],
                        rhs=xts[:, base + t, :],
                        start=(t == 0), stop=(t == SHOTS - 1))
                    if j == 0 and t == 0:
                        tile.add_dep_helper(
                            g_.ins, last_warm.ins,
                            reason="PE warm-up before stage S")
                nc.vector.tensor_copy(rpart[:, :, j], gp[:])

            # ---------------- AllReduce of Gram partials (bf16) -----------
            # One collective: per-op fixed cost (~12us RDH) makes splitting
            # counterproductive; the warm-up AllGather already absorbed the
            # CC-stream init.
            nc.sync.dma_start(cc_in[:, 60:C + 1, :], rpart[:, 60:C + 1, :])
            nc.scalar.dma_start(cc_in[:, 0:60, :], rpart[:, 0:60, :])
            nc.gpsimd.collective_compute(
                "AllReduce", ALU.add, replica_groups=groups,
                ins=[cc_in[:]], outs=[cc_out[:]],
            )
            # mean column (c'=C) first: the correction path depends on it.
            # Two pulls only — a gpsimd pull would drag its SWDGE drain
            # into the tail.
            for eng, d0, d1 in ((nc.sync, 64, C + 1), (nc.scalar, 0, 64)):
                eng.dma_start(rall[:, d0:d1, :], cc_out[:, d0:d1, :])

            # ---------------- stage Q: query norms ----------------
            for b in range(BLOC):
                sq = vtp.tile([C, HW], f32, tag="sq")
                nc.scalar.activation(sq[:], qnat[:, b, :], AF.Square,
                                     accum_out=nsq[:, b:b + 1])
            # rinv = nsq^(-1/2) by Newton from constant seed (nsq ~ 1024)
            r0 = 2.0 ** -5
            nc.vector.tensor_scalar(tnw[:], nsq[:], r0 * r0 * -0.5, 1.5,
                                    ALU.mult, ALU.add)
            nc.vector.tensor_scalar_mul(rin[:], tnw[:], r0)
            for _ in range(2):
                nc.vector.tensor_mul(tnw[:], rin[:], rin[:])
                nc.vector.tensor_mul(tnw[:], tnw[:], nsq[:])
                nc.vector.tensor_scalar(tnw[:], tnw[:], -0.5, 1.5,
                                        ALU.mult, ALU.add)
                nc.vector.tensor_mul(rin[:], rin[:], tnw[:])

            # wqT = qT * w'  (per-partition, per-chunk scale)
            for ci in range(QCH):
                nc.vector.tensor_scalar_mul(wqT[:, ci, :, :], qT[:, ci, :, :],
                                            wps[:, ci:ci + 1])

            # ---------------- stage W: W_b = diag(rin) W~_b diag(rin) -------
            for b in range(BLOC):
                wpt = w_ps.tile([C, C], f32, tag="wacc")
                for ci in range(QCH):
                    nc.tensor.matmul(wpt[:], lhsT=wqT[:, ci, b, :],
                                     rhs=qT[:, ci, b, :],
                                     start=(ci == 0), stop=(ci == QCH - 1))
                vt = vtp.tile([C, C], bf16, tag="vt")
                nc.scalar.activation(vt[:], wpt[:], AF.Copy,
                                     scale=rin[:, b:b + 1])
                pt = tp_ps.tile([C, C], bf16, tag="tp")
                nc.tensor.transpose(pt[:], vt[:], ident[:])
                nc.scalar.activation(wsb[:, 0:C, b], pt[:], AF.Copy,
                                     scale=rin[:, b:b + 1])

            # ---------------- mean-correction prep ----------------
            nc.scalar.activation(mallN[:], rall[:, C, :], AF.Copy,
                                 scale=-1.0 / NTOT)
            mt = tp_ps.tile([WAYS, C], bf16, tag="tp")
            nc.tensor.transpose(mt[:], rall[:, C, :], ident[:])
            nc.vector.tensor_copy(msT[:], mt[:])

            # ---------------- correction: -(1/N) m^T W_b m ----------------
            for h in range(2):
                up = w_ps.tile([WAYS, BLOC * C // 2], f32, tag="wacc")
                nc.tensor.matmul(up[:], lhsT=mallN[:],
                                 rhs=wsb[:, 64 * h:64 * (h + 1), :],
                                 start=True, stop=True)
                nc.vector.tensor_tensor(
                    ytmp[:, :, 64 * h:64 * (h + 1)],
                    up[:].rearrange("j (d b) -> j b d", b=BLOC),
                    msT[:, None, 64 * h:64 * (h + 1)].to_broadcast(
                        (WAYS, BLOC, C // 2)),
                    ALU.mult)
            nc.vector.tensor_reduce(ysb[:], ytmp[:],
                                    axis=mybir.AxisListType.X, op=ALU.add)

            # ---------------- Frobenius: score[j,b] = <R_j, W_b> -----------
            # k-packed: chunk t covers c-cols [12t, 12t+12); lhsT [128, 120]
            # = (ci,j) cols of R, rhs [128, 96] = (ci,b) cols of W.  All
            # chunks PSUM-accumulate; diagonal blocks (i,i) hold the score
            # partials.  Chunk order follows rall pull arrival (sync slice
            # [64:129) lands first, then scalar [0:64)).
            frp = fr_ps.tile([KP * WAYS, KP * BLOC], f32, tag="fr")
            order = [6, 7, 8, 9, 10, 5, 0, 1, 2, 3, 4]
            for n, t in enumerate(order):
                c0 = KP * t
                nc.tensor.matmul(
                    frp[:],
                    lhsT=rall[:, c0:c0 + KP, :].rearrange("c ci j -> c (ci j)"),
                    rhs=wsb[:, c0:c0 + KP, :].rearrange("c ci b -> c (ci b)"),
                    start=(n == 0), stop=(n == NFB - 1))
            # diag-block extraction: mask out cross blocks, reduce over the
            # ci' free axis, then fold the 12 partition blocks with one
            # selector matmul (all APs partition-base 0)
            nc.vector.tensor_tensor(
                masked[:], frp[:].rearrange("p (ci b) -> p ci b", b=BLOC),
                msk[:], ALU.mult)
            nc.vector.tensor_reduce(red[:],
                                    masked[:].rearrange("p ci b -> p b ci"),
                                    axis=mybir.AxisListType.X, op=ALU.add)
            finp = w_ps.tile([WAYS, BLOC], f32, tag="wacc")
            nc.tensor.matmul(finp[:], lhsT=sel[:], rhs=red[:],
                             start=True, stop=True)
            nc.vector.tensor_add(fin[:], finp[:], ysb[:])
            nc.sync.dma_start(out_d[:], fin[:])

    nc.compile()
    return nc


def _get_program():
    if "nc" not in _CACHE:
        _CACHE["nc"] = _build_program()
    return _CACHE["nc"]


def _make_in_maps(q, support, conv_w):
    bf = ml_dtypes.bfloat16
    q = np.asarray(q, dtype=np.float32).reshape(B, C, HW)
    sup = np.asarray(support, dtype=np.float32).reshape(WAYS, SHOTS, C, HW)
    w = np.ascontiguousarray(np.asarray(conv_w, dtype=np.float32))
    in_maps = []
    for k in range(NCORES):
        # support: [ways, shots, C, pix-slice] -> [p, (j t), C] + ones col
        s = sup[:, :, :, k * PIX:(k + 1) * PIX]
        xts = np.empty((128, CHUNKS, C + 1), dtype=bf)
        xts[:, :, 0:C] = s.transpose(3, 0, 1, 2).reshape(
            PIX, CHUNKS, C).astype(bf)
        xts[:, :, C] = bf(1.0)
        # q block: [8, C, HW]
        qb = q[k * BLOC:(k + 1) * BLOC]
        qT = np.ascontiguousarray(
            qb.reshape(BLOC, C, QCH, 128).transpose(3, 2, 0, 1)).astype(bf)
        qnat = np.ascontiguousarray(qb.transpose(1, 0, 2)).astype(bf)
        in_maps.append({
            "support": np.ascontiguousarray(xts),
            "q": qT,
            "qnat": qnat,
            "conv_w": w,
        })
    return in_maps


def _run(in_maps, trace=False):
    from concourse.bass_utils import run_bass_kernel_spmd
    nc = _get_program()
    return run_bass_kernel_spmd(nc, in_maps, list(range(NCORES)), trace=trace)


def kernel(q, support, conv_w):
    res = _run(_make_in_maps(q, support, conv_w))
    out = np.concatenate(
        [res.results[k]["out"].T for k in range(NCORES)], axis=0)
    return np.ascontiguousarray(out.astype(np.float32))


# revision 61
# speedup vs baseline: 1.1211x; 1.1211x over previous
"""Trainium2 Bass kernel for nn_Baseline_635655160228 (retrieval_knn).

Reference computation (B=64, WAYS=10, SHOTS=5, C=128, H=W=32):
    cov_j = centered-Gram(support_j) / (N-1)          # [ways, C, C], N = shots*hw
    qn    = q / ||q||_2(per channel row)              # [B, C, hw]
    sim[b,j,p] = qn_p^T cov_j qn_p                    # diag quadratic form
    out[b,j]   = sum_p leaky_relu(sim) * conv_w[p]

Algebraic restructuring:
  cov_j is PSD, so LeakyReLU is the identity, and
      out[b,j] = <cov_j, W_b>_F,   W_b = qn diag(w') qn^T  (w' = conv_w/(N-1))
  Since the channel-row normalization is a per-channel scale (qn = diag(rin) q),
  W_b = diag(rin_b) W~_b diag(rin_b) with W~_b built from RAW q.  W~ is
  symmetric, so the two-sided scale is: ACT-scale rows, PE-transpose,
  ACT-scale rows again.
  Mean correction applied at the end:
      out[b,j] = <R_j, W_b> - (1/N) m_j^T W_b m_j   (R raw Gram, m row sums).

Distribution over 8 NeuronCores:
  - data-parallel over the query batch (8 queries per core)
  - Grams sharded over the pixel axis (128-pixel slice per core), combined
    with one bf16 AllReduce of raw Gram + row sums, overlapped with all the
    query-side work.  A tiny warm-up AllReduce at t~0 absorbs comm-init.

Layout is precomputed on host (pure shard/cast/transpose, no math):
  - support arrives pre-transposed as xts [p=128, ways*shots, C+1] bf16 with a
    ones column (row sums fall out of the Gram matmul)
  - q arrives both as qT [p, ci, b, c] bf16 (for the W~ build) and natural
    [c, b, hw] bf16 (for the row norms)
  Host-side prep kills all 114 PE transposes and the per-way rearranging DMAs.

Frobenius stage is k-packed: 12 c-columns per matmul (lhsT [128, 120],
rhs [128, 96]), PSUM-accumulated over 11 chunks; the 12 diagonal [10,8]
blocks are summed on DVE/ACT.  11 matmuls instead of 128.
"""

import numpy as np
import ml_dtypes

B, WAYS, SHOTS, C, H, W = 64, 10, 5, 128, 32, 32
HW = H * W                       # 1024
NCORES = 8
BLOC = B // NCORES               # 8 queries per core
PIX = HW // NCORES               # 128-pixel support slice per core
NTOT = SHOTS * HW                # 5120 samples per way
DENOM = float(NTOT - 1)          # 5119
CHUNKS = WAYS * SHOTS            # 50 local [128, C] sample chunks
QCH = HW // 128                  # 8 pixel chunks per query
KP = 12                          # Frobenius column-pack factor
NFB = (C + KP - 1) // KP         # 11 packed Frobenius matmuls

_CACHE = {}


def _build_program():
    import concourse.bass as bass
    import concourse.tile as tile
    from concourse import bacc, mybir

    f32 = mybir.dt.float32
    bf16 = mybir.dt.bfloat16
    AF = mybir.ActivationFunctionType
    ALU = mybir.AluOpType

    nc = bacc.Bacc("TRN2", target_bir_lowering=False, debug=False,
                   num_devices=NCORES)

    # host-prepped inputs (bf16, pre-transposed layouts)
    xts_d = nc.dram_tensor("support", [128, CHUNKS, C + 1], bf16,
                           kind="ExternalInput")
    qT_d = nc.dram_tensor("q", [128, QCH, BLOC, C], bf16,
                          kind="ExternalInput")
    qnat_d = nc.dram_tensor("qnat", [C, BLOC, HW], bf16,
                            kind="ExternalInput")
    w_d = nc.dram_tensor("conv_w", [HW], f32, kind="ExternalInput")
    out_d = nc.dram_tensor("out", [WAYS, BLOC], f32, kind="ExternalOutput")

    # collective bounce buffers (transposed layout: [c, c'-col, way]).
    # fp8 e4m3 payload, prescaled by 1/32 (diag ~160, off-diag ~2.3 — well
    # inside e4m3's normal range); halves the AllReduce bytes.
    f8 = mybir.dt.float8e4
    cc_in = nc.dram_tensor("cc_in", [C, C + 1, WAYS], f8)
    cc_out = nc.dram_tensor("cc_out", [C, C + 1, WAYS], f8,
                            addr_space="Shared")
    # warm-up collective scratch: never initialized nor read — its only
    # purpose is to ring the CC doorbell as early as possible so the
    # CC-stream init / rendezvous runs under the local compute.  AllGather:
    # lowest-floor collective (~4.6us vs ~9.7us for AllReduce).
    ccw_in = nc.dram_tensor("ccw_in", [32, 8], bf16)
    ccw_out = nc.dram_tensor("ccw_out", [128, 8], bf16)
    groups = [list(range(NCORES))]

    with tile.TileContext(nc) as tc:
        # warm-up collective first — no data deps, doorbell rings at t~0.
        # 4-rank AllGather groups: shortest working warm-up flight.
        from concourse.mybir import AluOpType as _ALU
        nc.gpsimd.collective_compute(
            "AllGather", _ALU.bypass,
            replica_groups=[[0, 1, 2, 3], [4, 5, 6, 7]],
            ins=[ccw_in[:]], outs=[ccw_out[:]],
        )
        with (
            tc.tile_pool(name="const", bufs=1) as constp,
            tc.tile_pool(name="big", bufs=1) as big,
            tc.tile_pool(name="vt", bufs=2) as vtp,
            tc.tile_pool(name="tp_ps", bufs=2, space="PSUM") as tp_ps,
            tc.tile_pool(name="gram_ps", bufs=2, space="PSUM") as gram_ps,
            tc.tile_pool(name="w_ps", bufs=2, space="PSUM") as w_ps,
            tc.tile_pool(name="fr_ps", bufs=1, space="PSUM") as fr_ps,
        ):
            ident_d = nc.inline_tensor(
                np.eye(128, dtype=ml_dtypes.bfloat16), name="ident_const")
            ident = constp.tile([128, 128], bf16, tag="ident")

            # Frobenius diag-block mask M[(i,j),(i',b)] = (i == i') and the
            # partition-block fold selector SEL[(i,j), j] = 1
            msk_np = np.zeros((KP * WAYS, KP * BLOC), np.float32)
            sel_np = np.zeros((KP * WAYS, WAYS), np.float32)
            for i in range(KP):
                msk_np[i * WAYS:(i + 1) * WAYS,
                       i * BLOC:(i + 1) * BLOC] = 1.0
                for j in range(WAYS):
                    sel_np[i * WAYS + j, j] = 1.0
            msk_d = nc.inline_tensor(msk_np, name="fmask_const")
            msk = constp.tile([KP * WAYS, KP, BLOC], f32, tag="msk")
            sel_d = nc.inline_tensor(sel_np, name="sel_const")
            sel = constp.tile([KP * WAYS, WAYS], f32, tag="sel")

            wp = constp.tile([128, QCH], f32, tag="wp")        # conv_w p-major
            wps = constp.tile([128, QCH], f32, tag="wps")      # conv_w/(N-1)

            # ---------------- persistent tensors ----------------
            # R and W live in transposed layouts [c, c'-col, way/query] so the
            # k-packed Frobenius sees its (ci, j)/(ci, b) column groups as one
            # CONTIGUOUS free dim (matmul APs allow only one free dim).
            # Both are padded to KP*NFB=132 c'-cols (zeroed); rall's col C is
            # the mean column and pairs with wsb's zero pad -> contributes 0.
            CP = KP * NFB
            xts = big.tile([128, CHUNKS, C + 1], bf16, tag="xts")
            rpart = big.tile([C, C + 1, WAYS], f8, tag="rpart")
            rall = big.tile([C, CP, WAYS], bf16, tag="rall")
            qT = big.tile([128, QCH, BLOC, C], bf16, tag="qT")
            wqT = big.tile([128, QCH, BLOC, C], bf16, tag="wqT")
            qnat = big.tile([C, BLOC, HW], bf16, tag="qnat")
            wsb = big.tile([C, CP, BLOC], bf16, tag="wsb")

            nsq = constp.tile([128, BLOC], f32, tag="nsq")
            rin = constp.tile([128, BLOC], f32, tag="rin")
            tnw = constp.tile([128, BLOC], f32, tag="tnw")
            mallN = constp.tile([C, WAYS], bf16, tag="mallN")
            msT = constp.tile([WAYS, C], f32, tag="msT")
            ytmp = constp.tile([WAYS, BLOC, C], f32, tag="ytmp")
            ysb = constp.tile([WAYS, BLOC], f32, tag="ysb")
            masked = constp.tile([KP * WAYS, KP, BLOC], f32, tag="masked")
            red = constp.tile([KP * WAYS, BLOC], f32, tag="red")
            fin = constp.tile([WAYS, BLOC], f32, tag="fin")

            nc.vector.memset(rall[:, C + 1:CP, :], 0.0)
            nc.vector.memset(wsb[:, C:CP, :], 0.0)

            # ---------------- input DMAs ----------------
            # sync: ident + first half of xts; scalar: rest of xts + qT;
            # gpsimd (SWDGE): qnat.  Support path has priority end-to-end.
            nc.sync.dma_start(ident[:], ident_d[:])
            for i in range(3):
                j0, j1 = [(0, 10), (10, 20), (20, 25)][i]
                nc.sync.dma_start(xts[:, j0:j1, :], xts_d[:, j0:j1, :])
            for i in range(3):
                j0, j1 = [(25, 35), (35, 45), (45, 50)][i]
                nc.scalar.dma_start(xts[:, j0:j1, :], xts_d[:, j0:j1, :])
            for h in range(2):
                nc.gpsimd.dma_start(qnat[:, 4 * h:4 * h + 4, :],
                                    qnat_d[:, 4 * h:4 * h + 4, :])
            for h in range(2):
                nc.scalar.dma_start(qT[:, 4 * h:4 * h + 4, :, :],
                                    qT_d[:, 4 * h:4 * h + 4, :, :])
            nc.sync.dma_start(wp[:], w_d.rearrange("(ci p) -> p ci", p=128))
            # x32 undoes the fp8 collective prescale (1/32)
            nc.vector.tensor_scalar_mul(wps[:], wp[:], 32.0 / DENOM)
            nc.gpsimd.dma_start(msk[:], msk_d.rearrange("p (ci b) -> p ci b",
                                                        b=BLOC))
            nc.gpsimd.dma_start(sel[:], sel_d[:])

            # ---------------- PE warm-up (p-state ramp) ----------------
            warm = fr_ps.tile([128, 128], f32, tag="fr")
            last_warm = None
            for wi in range(16):
                last_warm = nc.tensor.matmul(
                    warm[:], lhsT=ident[:], rhs=ident[:],
                    start=(wi == 0), stop=(wi == 15))

            # ---------------- stage S: local support Grams ----------------
            for j in range(WAYS):
                base = SHOTS * j
                gp = gram_ps.tile([C, C + 1], f32, tag="gram")
                for t in range(SHOTS):
                    g_ = nc.tensor.matmul(
                        gp[:], lhsT=xts[:, base + t, 0:C],
                        rhs=xts[:, base + t, :],
                        start=(t == 0), stop=(t == SHOTS - 1))
                    if j == 0 and t == 0:
                        tile.add_dep_helper(
                            g_.ins, last_warm.ins,
                            reason="PE warm-up before stage S")
                nc.vector.tensor_scalar_mul(rpart[:, :, j], gp[:], 1.0 / 32.0)

            # ---------------- AllReduce of Gram partials (bf16) -----------
            # One collective: per-op fixed cost (~12us RDH) makes splitting
            # counterproductive; the warm-up AllGather already absorbed the
            # CC-stream init.
            nc.sync.dma_start(cc_in[:, 60:C + 1, :], rpart[:, 60:C + 1, :])
            nc.scalar.dma_start(cc_in[:, 0:60, :], rpart[:, 0:60, :])
            nc.gpsimd.collective_compute(
                "AllReduce", ALU.add, replica_groups=groups,
                ins=[cc_in[:]], outs=[cc_out[:]],
            )
            # mean column (c'=C) first: the correction path depends on it.
            # SWDGE (gpsimd) pulls: they cast fp8 -> bf16 during the DMA.
            for d0, d1 in ((64, C + 1), (0, 64)):
                nc.gpsimd.dma_start(rall[:, d0:d1, :], cc_out[:, d0:d1, :])

            # ---------------- stage Q: query norms ----------------
            for b in range(BLOC):
                sq = vtp.tile([C, HW], f32, tag="sq")
                nc.scalar.activation(sq[:], qnat[:, b, :], AF.Square,
                                     accum_out=nsq[:, b:b + 1])
            # rinv = nsq^(-1/2) by Newton from constant seed (nsq ~ 1024)
            r0 = 2.0 ** -5
            nc.vector.tensor_scalar(tnw[:], nsq[:], r0 * r0 * -0.5, 1.5,
                                    ALU.mult, ALU.add)
            nc.vector.tensor_scalar_mul(rin[:], tnw[:], r0)
            for _ in range(2):
                nc.vector.tensor_mul(tnw[:], rin[:], rin[:])
                nc.vector.tensor_mul(tnw[:], tnw[:], nsq[:])
                nc.vector.tensor_scalar(tnw[:], tnw[:], -0.5, 1.5,
                                        ALU.mult, ALU.add)
                nc.vector.tensor_mul(rin[:], rin[:], tnw[:])

            # wqT = qT * w'  (per-partition, per-chunk scale)
            for ci in range(QCH):
                nc.vector.tensor_scalar_mul(wqT[:, ci, :, :], qT[:, ci, :, :],
                                            wps[:, ci:ci + 1])

            # ---------------- stage W: W_b = diag(rin) W~_b diag(rin) -------
            for b in range(BLOC):
                wpt = w_ps.tile([C, C], f32, tag="wacc")
                for ci in range(QCH):
                    nc.tensor.matmul(wpt[:], lhsT=wqT[:, ci, b, :],
                                     rhs=qT[:, ci, b, :],
                                     start=(ci == 0), stop=(ci == QCH - 1))
                vt = vtp.tile([C, C], bf16, tag="vt")
                nc.scalar.activation(vt[:], wpt[:], AF.Copy,
                                     scale=rin[:, b:b + 1])
                pt = tp_ps.tile([C, C], bf16, tag="tp")
                nc.tensor.transpose(pt[:], vt[:], ident[:])
                nc.scalar.activation(wsb[:, 0:C, b], pt[:], AF.Copy,
                                     scale=rin[:, b:b + 1])

            # ---------------- mean-correction prep ----------------
            # -32/N: mallN/msT both carry m/32, wsb carries an extra x32
            nc.scalar.activation(mallN[:], rall[:, C, :], AF.Copy,
                                 scale=-32.0 / NTOT)
            mt = tp_ps.tile([WAYS, C], bf16, tag="tp")
            nc.tensor.transpose(mt[:], rall[:, C, :], ident[:])
            nc.vector.tensor_copy(msT[:], mt[:])

            # ---------------- correction: -(1/N) m^T W_b m ----------------
            for h in range(2):
                up = w_ps.tile([WAYS, BLOC * C // 2], f32, tag="wacc")
                nc.tensor.matmul(up[:], lhsT=mallN[:],
                                 rhs=wsb[:, 64 * h:64 * (h + 1), :],
                                 start=True, stop=True)
                nc.vector.tensor_tensor(
                    ytmp[:, :, 64 * h:64 * (h + 1)],
                    up[:].rearrange("j (d b) -> j b d", b=BLOC),
                    msT[:, None, 64 * h:64 * (h + 1)].to_broadcast(
                        (WAYS, BLOC, C // 2)),
                    ALU.mult)
            nc.vector.tensor_reduce(ysb[:], ytmp[:],
                                    axis=mybir.AxisListType.X, op=ALU.add)

            # ---------------- Frobenius: score[j,b] = <R_j, W_b> -----------
            # k-packed: chunk t covers c-cols [12t, 12t+12); lhsT [128, 120]
            # = (ci,j) cols of R, rhs [128, 96] = (ci,b) cols of W.  All
            # chunks PSUM-accumulate; diagonal blocks (i,i) hold the score
            # partials.  Chunk order follows rall pull arrival (sync slice
            # [64:129) lands first, then scalar [0:64)).
            frp = fr_ps.tile([KP * WAYS, KP * BLOC], f32, tag="fr")
            order = [6, 7, 8, 9, 10, 5, 0, 1, 2, 3, 4]
            for n, t in enumerate(order):
                c0 = KP * t
                nc.tensor.matmul(
                    frp[:],
                    lhsT=rall[:, c0:c0 + KP, :].rearrange("c ci j -> c (ci j)"),
                    rhs=wsb[:, c0:c0 + KP, :].rearrange("c ci b -> c (ci b)"),
                    start=(n == 0), stop=(n == NFB - 1))
            # diag-block extraction: mask out cross blocks, reduce over the
            # ci' free axis, then fold the 12 partition blocks with one
            # selector matmul (all APs partition-base 0)
            nc.vector.tensor_tensor(
                masked[:], frp[:].rearrange("p (ci b) -> p ci b", b=BLOC),
                msk[:], ALU.mult)
            nc.vector.tensor_reduce(red[:],
                                    masked[:].rearrange("p ci b -> p b ci"),
                                    axis=mybir.AxisListType.X, op=ALU.add)
            finp = w_ps.tile([WAYS, BLOC], f32, tag="wacc")
            nc.tensor.matmul(finp[:], lhsT=sel[:], rhs=red[:],
                             start=True, stop=True)
            nc.vector.tensor_add(fin[:], finp[:], ysb[:])
            nc.sync.dma_start(out_d[:], fin[:])

    nc.compile()
    return nc


def _get_program():
    if "nc" not in _CACHE:
        _CACHE["nc"] = _build_program()
    return _CACHE["nc"]


def _make_in_maps(q, support, conv_w):
    bf = ml_dtypes.bfloat16
    q = np.asarray(q, dtype=np.float32).reshape(B, C, HW)
    sup = np.asarray(support, dtype=np.float32).reshape(WAYS, SHOTS, C, HW)
    w = np.ascontiguousarray(np.asarray(conv_w, dtype=np.float32))
    in_maps = []
    for k in range(NCORES):
        # support: [ways, shots, C, pix-slice] -> [p, (j t), C] + ones col
        s = sup[:, :, :, k * PIX:(k + 1) * PIX]
        xts = np.empty((128, CHUNKS, C + 1), dtype=bf)
        xts[:, :, 0:C] = s.transpose(3, 0, 1, 2).reshape(
            PIX, CHUNKS, C).astype(bf)
        xts[:, :, C] = bf(1.0)
        # q block: [8, C, HW]
        qb = q[k * BLOC:(k + 1) * BLOC]
        qT = np.ascontiguousarray(
            qb.reshape(BLOC, C, QCH, 128).transpose(3, 2, 0, 1)).astype(bf)
        qnat = np.ascontiguousarray(qb.transpose(1, 0, 2)).astype(bf)
        in_maps.append({
            "support": np.ascontiguousarray(xts),
            "q": qT,
            "qnat": qnat,
            "conv_w": w,
        })
    return in_maps


def _run(in_maps, trace=False):
    from concourse.bass_utils import run_bass_kernel_spmd
    nc = _get_program()
    return run_bass_kernel_spmd(nc, in_maps, list(range(NCORES)), trace=trace)


def kernel(q, support, conv_w):
    res = _run(_make_in_maps(q, support, conv_w))
    out = np.concatenate(
        [res.results[k]["out"].T for k in range(NCORES)], axis=0)
    return np.ascontiguousarray(out.astype(np.float32))
